# revision 11
# baseline (speedup 1.0000x reference)
"""Trainium2 Bass kernel for multi-head attention (B=2, S=2048, D=1024, H=16, causal, RoPE).

Sharding: hybrid batch x head tensor-parallel. Core c handles batch c//4 and
head group c%4 (4 heads = 256 of the 1024 q/k/v dims), processed as two
head-pairs (hp=0,1) of 128 dims each: QKV projections for its slice, RoPE,
causal attention, and a partial output projection against its 256-column
slice of o_weight. The host sums the 4 partial outputs per batch (the
all-reduce). vs. pure head-parallel this halves the per-core x input, the
bf16 partial output (and so the PSUM-evacuation copies and output DMA), and
the host reduction.

Device-side layout choices:
  - QKV projections run as fp8e4 DoubleRow matmuls (2 contraction rows per
    partition, half cost per the PE model): x and the weights arrive as
    host-prepared fp8 hi+lo pairs (weights pre-scaled by 32 so fp8 normals
    cover them; the 32*32 factor is folded into the exp scale and into wo).
    v uses full hi*hi + hi*lo + lo*hi compensation (~0.1% error); q/k drop
    the x_lo term (2-term) - the induced error is suppressed by the small
    score magnitudes and lands at rel_err ~1.75e-2 (budget 2e-2).
  - Activations live transposed: q/k are [128 (head dims), seq] so the
    scores matmul contracts dh on partitions. RoPE pairs are de-interleaved
    on the host (weight-row permutation) so pair partners sit 32 partitions
    apart; the rotate step is a single 128x128 sign-swap matmul (sperm).
  - Scores also run as fp8e4 DoubleRow at half cost: the stationary pair is
    (fp8(k), k - fp8(k)) so the k-side quantization error cancels; the
    moving q8 is a stride-0 broadcast over the pair dim. Scores are
    computed transposed ([sk, sq]) so P = exp(scores) feeds the PV matmul
    directly as the moving bf16 operand. V carries a block of 64 ones
    columns so the PV matmul also emits the softmax denominator;
    normalization is a plain reciprocal+multiply.
  - V is projected directly into [seq, dh] layout by using the (transposed)
    x tiles as the stationary operand - no on-chip transposes needed.
  - Work is software-pipelined: projection chunks run one chunk ahead of
    attention, output projections trail one chunk behind (they need both
    head-pairs' normalized outputs), both threaded through the attention
    tile loop. Dummy matmuls at t=0 cover the first DMAs and pre-ramp the
    PE p-state clock. Engine assignment of the PSUM-evacuation copies
    (ACT/DVE only - GPSIMD cannot touch PSUM) and the scheduling knobs
    below were tuned by timeline-simulator sweep.
  - Only one PSUM accumulation group may live per 2KB bank (zero region),
    so accumulators are bank-aligned and column-split starts are avoided.
  - Partial outputs are written in bf16 (summed in fp32 on the host).
"""

import numpy as np

D_MODEL = 1024
N_HEADS = 16
D_HEAD = 64
THETA = 10000.0
B = 2
S = 2048          # per-core sequence (one batch)
N_CORES = 8
NQ = 512          # query chunk width
NK = 128          # key tile width
NC = S // NQ      # 4 chunks
HP = 2            # head-pairs per core (4 heads)

N_DUMMY = 13
PV_DEPTH_N = 3
TMPQ_DVE = False
KRES = True
KXLO = False        # k-projection x_lo term (2-term: rel_err ~1.75e-2)
QXLO = False        # q-projection x_lo term
# oproj mid quota per chunk-pass (c,hp) = (0,0),(0,1),(1,0)..(3,1);
# leftovers drain after the loop
OPROJ_Q = [0, 0, 0, 2, 2, 2, 3, 3]
OPROJ_PA = True     # final-chunk oproj uses the idle pa PSUM ring
OPROJ_PA_ACT = False
TAIL_CB = True      # interleave final-chunk oproj with per-tile normalize
ROPE_EARLY_DVE = True  # early chunks: rope k-mul on DVE to unload Pool
ROPE_EARLY_U = 2
TMP_ENG = {"q": "dve", "k": "act"}  # per-proj tmp-copy engine
OPROJ_HALF = True
YS_MODE = 1
MERGE_V2 = False
PP_BUFS = 5
PYS_BUFS = 8
PX_BUFS = 4
PTMP_BUFS = 4

_RT = {}


def _build():
    if _RT:
        return _RT
    import sys
    try:
        import concourse.bass  # noqa: F401
    except ImportError:
        sys.path.insert(0, "/opt/trn_rl_repo")
    import concourse.mybir as mybir
    import concourse.tile as tile
    from concourse import bacc
    from concourse._compat import axon_active
    from concourse.bass_utils import run_bass_kernel_spmd

    f32 = mybir.dt.float32
    f32r = mybir.dt.float32r
    bf16 = mybir.dt.bfloat16
    fp8 = mybir.dt.float8e4
    EXP = mybir.ActivationFunctionType.Exp
    DR = mybir.MatmulPerfMode.DoubleRow

    nc = bacc.Bacc(
        "TRN2", target_bir_lowering=False, debug=not axon_active(),
        num_devices=N_CORES,
    )

    xh8 = nc.dram_tensor("xh8", [D_MODEL, S], fp8, kind="ExternalInput").ap()
    xl8 = nc.dram_tensor("xl8", [D_MODEL, S], fp8, kind="ExternalInput").ap()
    wq8 = nc.dram_tensor("wq8", [128, 2, 4, 2, 256], fp8, kind="ExternalInput").ap()
    wk8 = nc.dram_tensor("wk8", [128, 2, 4, 2, 256], fp8, kind="ExternalInput").ap()
    wv8 = nc.dram_tensor("wv8", [128, 2, 4, 2, 256], fp8, kind="ExternalInput").ap()
    wo = nc.dram_tensor("wo", [128, 2, D_MODEL], f32r, kind="ExternalInput").ap()
    trig = nc.dram_tensor("trig", [128, 2, S], bf16, kind="ExternalInput").ap()
    sperm = nc.dram_tensor("sperm", [128, 128], f32r, kind="ExternalInput").ap()
    mask128 = nc.dram_tensor("mask128", [128, 128], bf16, kind="ExternalInput").ap()
    y = nc.dram_tensor("y", [S, D_MODEL], bf16, kind="ExternalOutput").ap()

    with tile.TileContext(nc) as tc:
        with (
            tc.tile_pool(name="singles", bufs=1) as singles,
            tc.tile_pool(name="px", bufs=PX_BUFS) as px,
            tc.tile_pool(name="ptmp", bufs=PTMP_BUFS) as ptmp,
            tc.tile_pool(name="pp", bufs=PP_BUFS) as pp,
            tc.tile_pool(name="pys", bufs=PYS_BUFS) as pys,
            tc.tile_pool(name="pr", bufs=3) as pr,
            tc.tile_pool(name="ps_a", bufs=2, space="PSUM") as ps_a,
            tc.tile_pool(name="ps_s", bufs=2, space="PSUM") as ps_s,
            tc.tile_pool(name="ps_o", bufs=2, space="PSUM") as ps_o,
        ):
            wq_sb = singles.tile([128, 2, 4, 2, 256], fp8, tag="wq")
            wk_sb = singles.tile([128, 2, 4, 2, 256], fp8, tag="wk")
            wv_sb = singles.tile([128, 2, 4, 2, 256], fp8, tag="wv")
            wo_sb = singles.tile([128, 2, D_MODEL], f32r, tag="wo")
            sperm_sb = singles.tile([128, 128], f32r, tag="sperm")
            m128_sb = singles.tile([128, 128], bf16, tag="m128")
            trig_sb = singles.tile([128, 2, S], bf16, tag="trig")
            warm_sb = singles.tile([1, 2], f32, tag="warm")
            # q8: rope'd q in fp8, [dims, hp, chunk, q]
            q8_sb = singles.tile([128, HP, NC, NQ], fp8, tag="q8")
            # k8: [head dims, hp, (fp8(k), k-fp8(k)), key pos]
            k8_sb = singles.tile([128, HP, 2, S], fp8, tag="k8")
            # oT: [dims, hp, q] normalized attention out
            oT_sb = singles.tile([128, HP, S], f32r, tag="oT")
            # V tiles: [seq-tile partitions, hp, 16 tiles, 192]: cols 0:64
            # head A, 64:128 ones, 128:192 head B.
            v_sb = singles.tile([128, HP, 16, 192], bf16, tag="v")

            junk_sb = singles.tile([128, 512], f32r, tag="junk")

            nc.scalar.dma_start(out=wq_sb, in_=wq8)
            nc.scalar.dma_start(out=trig_sb[:, :, 0:2 * NQ],
                                in_=trig[:, :, 0:2 * NQ])
            nc.scalar.dma_start(out=wk_sb, in_=wk8)
            nc.scalar.dma_start(out=sperm_sb, in_=sperm)
            nc.scalar.dma_start(out=wv_sb, in_=wv8)
            nc.scalar.dma_start(out=m128_sb, in_=mask128)
            nc.vector.memset(v_sb[:, :, :, 64:128].bitcast(bf16), 1.0)

            # warm the ACT exp table before the first real exp
            nc.vector.memset(warm_sb, 0.0)
            nc.scalar.activation(warm_sb[:, 0:1], warm_sb[:, 1:2], EXP)

            # dummy matmuls: keep PE busy during the initial DMAs and ramp
            # the p-state clock before the first real matmul arrives
            nc.gpsimd.memset(junk_sb.bitcast(f32), 0.0)
            for w in range(N_DUMMY):
                jp = ps_s.tile([128, 2, 512], f32, tag="sps")
                nc.tensor.matmul(jp[:, 0, :], junk_sb[:, 0:128], junk_sb,
                                 start=True, stop=True)

            def late_consts():
                nc.sync.dma_start(out=wo_sb, in_=wo)

            xt_cache = {}

            def load_x(c):
                s0 = NQ * c
                xt = [None] * 4
                for lo in range(2):
                    for h in range(2):
                        xth = px.tile([128, 4, 512], fp8, tag=f"xt{lo}{h}",
                                      name="xth")
                        nc.sync.dma_start(
                            out=xth,
                            in_=(xh8 if lo == 0 else xl8)
                            [512 * h:512 * (h + 1), s0:s0 + NQ]
                            .rearrange("(a p) n -> p a n", p=128),
                        )
                        xt[2 * lo + h] = xth
                xt_cache[c] = xt

            def proj_pieces(c):
                """QKV projections + rope + V transpose for seq chunk c
                (512 positions, both head-pairs), as a list of closures that
                can be threaded through the attention tile loop. x tiles are
                preloaded via load_x()."""
                s0 = NQ * c
                xt = xt_cache[c]
                tmps = {}

                def xpair(lo, j):
                    # [128, 2, 512] fp8: K-chunk pair (2j, 2j+1)
                    h, a = divmod(2 * j, 4)
                    return xt[2 * lo + h][:, a:a + 2, :]

                def p_proj(w_sb, name, hp):
                    # DoubleRow fp8, hi/lo compensated
                    def f():
                        hsl = slice(128 * hp, 128 * (hp + 1))
                        ps = ps_a.tile([128, 512], f32, tag="pa")
                        mm = [(0, 0, j) for j in range(4)] + \
                             [(1, 0, j) for j in range(4)]
                        if (name == "q" and QXLO) or (name == "k" and KXLO):
                            mm += [(0, 1, j) for j in range(4)]
                        for n, (wl, xl, j) in enumerate(mm):
                            nc.tensor.matmul(
                                ps, w_sb[:, wl, j, :, hsl], xpair(xl, j),
                                start=(n == 0), stop=(n == len(mm) - 1),
                                perf_mode=DR,
                            )
                        tmp = ptmp.tile([128, 512], f32r, tag=f"{name}tmp{hp}")
                        mode = TMP_ENG.get(name, "act")
                        if mode == "dve" or (name == "q" and TMPQ_DVE):
                            nc.vector.tensor_copy(tmp, ps)  # PSUM -> SBUF
                        elif mode == "split":
                            nc.vector.tensor_copy(tmp[:, 0:256], ps[:, 0:256])
                            nc.scalar.copy(tmp[:, 256:512], ps[:, 256:512])
                        else:
                            nc.scalar.copy(tmp, ps)  # GPSIMD can't read PSUM
                        tmps[(name, hp)] = tmp
                    return f

                def p_rope_q(hp):
                    def f():
                        tmp = tmps[("q", hp)]
                        sq = ps_a.tile([128, 512], f32, tag="pa")
                        nc.tensor.matmul(sq, sperm_sb, tmp, start=True,
                                         stop=True)
                        cs = trig_sb[:, 0, s0:s0 + NQ]
                        sn = trig_sb[:, 1, s0:s0 + NQ]
                        m1 = ptmp.tile([128, 512], f32, tag="m1")
                        m2 = ptmp.tile([128, 512], f32, tag="m2")
                        nc.gpsimd.tensor_mul(m1, tmp, cs)
                        nc.vector.tensor_mul(m2, sq, sn)
                        nc.gpsimd.tensor_add(q8_sb[:, hp, c, :], m1, m2)
                    return f

                def p_rope_k(hp):
                    def f():
                        tmp = tmps[("k", hp)]
                        sq = ps_a.tile([128, 512], f32, tag="pa")
                        nc.tensor.matmul(sq, sperm_sb, tmp, start=True,
                                         stop=True)
                        cs = trig_sb[:, 0, s0:s0 + NQ]
                        sn = trig_sb[:, 1, s0:s0 + NQ]
                        m1 = ptmp.tile([128, 512], f32, tag="m1")
                        k8h = k8_sb[:, hp, 0, s0:s0 + NQ]
                        if not KRES:
                            m2 = ptmp.tile([128, 512], f32, tag="m2")
                            nc.gpsimd.tensor_mul(m1, tmp, cs)
                            nc.vector.tensor_mul(m2, sq, sn)
                            nc.gpsimd.tensor_add(k8h, m1, m2)
                            return
                        kf = ptmp.tile([128, 512], f32, tag="kf")
                        if ROPE_EARLY_DVE and c <= ROPE_EARLY_U:
                            nc.vector.tensor_mul(m1, tmp, cs)
                        else:
                            nc.gpsimd.tensor_mul(m1, tmp, cs)
                        nc.vector.tensor_mul(kf, sq, sn)
                        nc.gpsimd.tensor_add(kf, m1, kf)
                        nc.vector.tensor_copy(k8h, kf)
                        nc.gpsimd.tensor_sub(k8_sb[:, hp, 1, s0:s0 + NQ],
                                             kf, k8h)
                    return f

                vps = {}

                def p_vproj(hp, sub):
                    def f():
                        if hp not in vps:
                            vps[hp] = ps_a.tile([128, 4, 128], f32, tag="pa",
                                                name="vps")
                        hsl = slice(128 * hp, 128 * (hp + 1))
                        ss = slice(128 * sub, 128 * (sub + 1))
                        mm = [(0, 0, j) for j in range(4)] + \
                             [(0, 1, j) for j in range(4)] + \
                             [(1, 0, j) for j in range(4)]
                        for n, (wl, xl, j) in enumerate(mm):
                            nc.tensor.matmul(
                                vps[hp][:, sub, :],
                                xpair(xl, j)[:, :, ss],
                                wv_sb[:, wl, j, :, hsl],
                                start=(n == 0), stop=(n == len(mm) - 1),
                                perf_mode=DR,
                            )
                    return f

                def p_vstore_a(hp):
                    def f():
                        nc.vector.tensor_copy(
                            v_sb[:, hp, 4 * c:4 * c + 4, 0:64],
                            vps[hp][:, :, 0:64])
                    return f

                def p_vstore_b(hp):
                    def f():
                        nc.vector.tensor_copy(
                            v_sb[:, hp, 4 * c:4 * c + 4, 128:192],
                            vps[hp][:, :, 64:128])
                    return f

                pieces = []
                for hp in range(HP):
                    pieces += [p_proj(wq_sb, "q", hp), p_proj(wk_sb, "k", hp),
                               p_rope_q(hp), p_vproj(hp, 0), p_vproj(hp, 1),
                               p_rope_k(hp), p_vproj(hp, 2), p_vproj(hp, 3),
                               p_vstore_a(hp), p_vstore_b(hp)]
                return pieces

            def proj_chunk(c):
                for f in proj_pieces(c):
                    f()

            def oproj_piece(c, s4, late=False, use_pa=False):
                """Output projection for one 128-row seq tile; contracts both
                head-pairs' oT (256 dims = 2 accumulating matmuls)."""
                row0 = NQ * c + 128 * s4
                if use_pa and OPROJ_PA:
                    yp0 = ps_a.tile([128, 512], f32, tag="pa", name="yp0")
                    yp1 = ps_a.tile([128, 512], f32, tag="pa", name="yp1")
                    yph = [yp0, yp1]
                    for hn in range(2):
                        for hp in range(HP):
                            nc.tensor.matmul(
                                yph[hn],
                                oT_sb[:, hp, row0:row0 + 128],
                                wo_sb[:, hp, 512 * hn:512 * (hn + 1)],
                                start=(hp == 0), stop=(hp == HP - 1),
                            )
                    ys = pys.tile([128, 1024], bf16, tag="ys")
                    if late == "drain":
                        nc.scalar.copy(ys[:, 0:512], yp0)
                        nc.vector.tensor_copy(ys[:, 512:1024], yp1)
                    elif OPROJ_PA_ACT:
                        nc.scalar.copy(ys[:, 0:512], yp0)
                        nc.scalar.copy(ys[:, 512:1024], yp1)
                    else:
                        nc.vector.tensor_copy(ys[:, 0:512], yp0)
                        nc.vector.tensor_copy(ys[:, 512:1024], yp1)
                    nc.sync.dma_start(out=y[row0:row0 + 128, :], in_=ys)
                    return
                yp = ps_s.tile([128, 2, 512], f32, tag="sps")
                for hn in range(2):
                    for hp in range(HP):
                        nc.tensor.matmul(
                            yp[:, hn, :],
                            oT_sb[:, hp, row0:row0 + 128],
                            wo_sb[:, hp, 512 * hn:512 * (hn + 1)],
                            start=(hp == 0), stop=(hp == HP - 1),
                        )
                ys = pys.tile([128, 1024], bf16, tag="ys")
                if YS_MODE == 2 or (YS_MODE == 3 and late):
                    nc.vector.tensor_copy(ys[:, 0:512], yp[:, 0, :])
                    nc.scalar.copy(ys[:, 512:1024], yp[:, 1, :])
                elif YS_MODE == 0 or YS_MODE == 3:
                    if s4 % 2 == 0:
                        nc.vector.tensor_copy(ys, yp.rearrange("p a n -> p (a n)"))
                    else:
                        nc.scalar.copy(ys, yp.rearrange("p a n -> p (a n)"))
                elif late == "tail":
                    nc.vector.tensor_copy(ys[:, 0:512], yp[:, 0, :])
                    nc.vector.tensor_copy(ys[:, 512:1024], yp[:, 1, :])
                elif late or s4 % 2 == 0:
                    nc.vector.tensor_copy(ys, yp.rearrange("p a n -> p (a n)"))
                else:
                    nc.scalar.copy(ys, yp.rearrange("p a n -> p (a n)"))
                nc.sync.dma_start(out=y[row0:row0 + 128, :], in_=ys)

            def attn_chunk(hp, c, mids=(), fine_tail=False, tail_cb=None):
                """Causal attention for query chunk c, head-pair hp. ``mids``
                are emitted one per attention tile (pipelined filler work)."""
                mids = list(mids)
                qsl = slice(NQ * c, NQ * (c + 1))
                nt = (NQ // NK) * (c + 1)
                oa = ps_o.tile([128, 512], f32, tag="oacc")
                ob = ps_o.tile([128, 512], f32, tag="oacc")
                pending = []  # (p tile, j, t) awaiting PV matmul
                PV_DEPTH = PV_DEPTH_N

                def pv_flush():
                    p, j, _t = pending.pop(0)
                    w0 = 128 * j
                    nc.tensor.matmul(
                        oa[:, w0:512], v_sb[:, hp, _t, 0:128],
                        p[:, 0, w0:512],
                        start=(_t == 0), stop=(_t == nt - 1),
                    )
                    nc.tensor.matmul(
                        ob[:, w0:512], v_sb[:, hp, _t, 64:192],
                        p[:, 1, w0:512],
                        start=(_t == 0), stop=(_t == nt - 1),
                    )

                for t in range(nt):
                    j = max(0, t - 4 * c)  # within-chunk diagonal offset
                    w0 = 128 * j           # causally-dead query columns
                    sps = ps_s.tile([128, 2, 512], f32, tag="sps")
                    for h in range(2):
                        hs = slice(64 * h, 64 * h + 64)
                        nc.tensor.matmul(
                            sps[:, h, w0:512],
                            k8_sb[hs, hp, :, 128 * t:128 * (t + 1)],
                            q8_sb[hs, hp, c, w0:512].unsqueeze(1)
                            .to_broadcast([64, 2, 512 - w0]),
                            start=True, stop=True, perf_mode=DR,
                        )
                    p = pp.tile([128, 2, 512], bf16, tag="p")
                    nc.scalar.activation(
                        p[:, :, w0:512], sps[:, :, w0:512], EXP,
                        scale=0.125 / 1024.0,
                    )
                    if t >= 4 * c:  # diagonal tile: mask boundary block
                        pb = p[:, :, w0:w0 + 128]
                        nc.vector.tensor_mul(
                            pb, pb,
                            m128_sb.unsqueeze(1).to_broadcast([128, 2, 128]),
                        )
                    if len(pending) >= PV_DEPTH:
                        pv_flush()
                    pending.append((p, j, t))
                    if mids:
                        mids.pop(0)()
                while pending:
                    pv_flush()
                for m in mids:  # in case nt < len(mids)
                    m()

                # oa rows 64:128 / ob rows 0:64 hold the replicated
                # softmax denominators (from the ones block in V).
                rra = pr.tile([64, 512], f32, tag="rra")
                rrb = pr.tile([64, 512], f32, tag="rrb")
                if fine_tail:
                    for s4 in range(4):
                        fs = slice(128 * s4, 128 * (s4 + 1))
                        qs4 = slice(qsl.start + 128 * s4,
                                    qsl.start + 128 * (s4 + 1))
                        nc.vector.reciprocal(rra[:, fs], oa[64:128, fs])
                        nc.vector.tensor_mul(oT_sb[0:64, hp, qs4],
                                             oa[0:64, fs], rra[:, fs])
                        nc.vector.reciprocal(rrb[:, fs], ob[0:64, fs])
                        nc.vector.tensor_mul(oT_sb[64:128, hp, qs4],
                                             ob[64:128, fs], rrb[:, fs])
                        if tail_cb is not None:
                            tail_cb(s4)
                else:
                    nc.vector.reciprocal(rra, oa[64:128, :])
                    nc.vector.reciprocal(rrb, ob[0:64, :])
                    nc.vector.tensor_mul(oT_sb[0:64, hp, qsl], oa[0:64, :],
                                         rra)
                    nc.vector.tensor_mul(oT_sb[64:128, hp, qsl],
                                         ob[64:128, :], rrb)

            # Software pipeline: projections run one chunk ahead of attention
            # (threaded through both head-pair passes of the previous chunk);
            # output projections trail their chunk's second head-pair pass.
            owed = []  # (c, s4) oproj pieces not yet emitted
            next_pieces = [[]]
            load_x(0)
            load_x(1)
            proj_chunk(0)
            # deferred loads, threaded into early attention passes:
            # (pass u=0): wo; (u=1): x(2) + trig tail; (u=3): x(3)
            def late_loads(u):
                if u == 0:
                    return [late_consts]
                if u == 1:
                    return [lambda: load_x(2),
                            lambda: nc.scalar.dma_start(
                                out=trig_sb[:, :, 2 * NQ:S],
                                in_=trig[:, :, 2 * NQ:S])]
                if u == 3:
                    return [lambda: load_x(3)]
                return []
            for c in range(NC):
                for hp in range(HP):
                    u = HP * c + hp
                    last = c == NC - 1 and hp == HP - 1
                    pa_ = last
                    quota = OPROJ_Q[u]
                    take, owed = owed[:quota], owed[quota:]
                    mids = []
                    for t in take:
                        if pa_ and OPROJ_HALF:
                            def mk(t=t):
                                st = {}
                                pool_, tag_ = ((ps_a, "pa") if pa_
                                               else (ps_s, "sps"))

                                def f1():
                                    row0 = NQ * t[0] + 128 * t[1]
                                    yp0 = pool_.tile([128, 512], f32,
                                                     tag=tag_, name="yp0")
                                    for hp_ in range(HP):
                                        nc.tensor.matmul(
                                            yp0,
                                            oT_sb[:, hp_, row0:row0 + 128],
                                            wo_sb[:, hp_, 0:512],
                                            start=(hp_ == 0),
                                            stop=(hp_ == HP - 1))
                                    ys = pys.tile([128, 1024], bf16, tag="ys")
                                    nc.vector.tensor_copy(ys[:, 0:512], yp0)
                                    st["ys"] = ys

                                def f2():
                                    row0 = NQ * t[0] + 128 * t[1]
                                    yp1 = pool_.tile([128, 512], f32,
                                                     tag=tag_, name="yp1")
                                    for hp_ in range(HP):
                                        nc.tensor.matmul(
                                            yp1,
                                            oT_sb[:, hp_, row0:row0 + 128],
                                            wo_sb[:, hp_, 512:1024],
                                            start=(hp_ == 0),
                                            stop=(hp_ == HP - 1))
                                    ys = st["ys"]
                                    nc.vector.tensor_copy(ys[:, 512:1024], yp1)
                                    nc.sync.dma_start(
                                        out=y[row0:row0 + 128, :], in_=ys)
                                return [f1, f2]
                            mids += mk()
                        else:
                            late_ = c >= 2
                            mids.append(
                                (lambda t=t, late_=late_: oproj_piece(
                                    t[0], t[1], late=late_, use_pa=pa_)))
                    # thread the next chunk's projections through both of
                    # this chunk's head-pair passes: the load + hp0 pieces
                    # through the hp0 pass, the hp1 pieces through hp1
                    if hp == 0 and c + 1 < NC:
                        allp = proj_pieces(c + 1)
                        nsplit = 1 + (len(allp) - 1) // 2
                        pieces, next_pieces[0] = allp[:nsplit], allp[nsplit:]
                    elif hp == 1:
                        pieces, next_pieces[0] = next_pieces[0], []
                    else:
                        pieces = []
                    merged = late_loads(u)
                    while pieces or mids:
                        if pieces:
                            merged.append(pieces.pop(0))
                        if mids:
                            merged.append(mids.pop(0))
                    mids = merged
                    cb = None
                    if last and TAIL_CB:
                        cb = (lambda s4: oproj_piece(c, s4, late="tail",
                                                     use_pa=True))
                    attn_chunk(hp, c, mids=mids, fine_tail=last,
                               tail_cb=cb)
                    if hp == HP - 1 and not (last and cb is not None):
                        owed += [(c, s4) for s4 in range(4)]
            for t in owed:
                oproj_piece(t[0], t[1], late="drain", use_pa=True)

    nc.compile()
    _RT.update(
        nc=nc, run_bass_kernel_spmd=run_bass_kernel_spmd, mybir=mybir,
    )
    return _RT


def _host_inputs(q_weight, k_weight, v_weight, o_weight, in_features):
    """Build the per-core input maps (host-side sharding + layout prep)."""
    x = np.ascontiguousarray(np.asarray(in_features, dtype=np.float32))
    qw = np.asarray(q_weight, dtype=np.float32)
    kw = np.asarray(k_weight, dtype=np.float32)
    vw = np.asarray(v_weight, dtype=np.float32)
    ow = np.asarray(o_weight, dtype=np.float32)

    import ml_dtypes
    FP8 = ml_dtypes.float8_e4m3fn

    def w8pair(w):
        # [1024, 256] -> [128(part), 2(hi/lo), 4(pair), 2(sub), 256(col)]
        w32 = w * 32.0
        hi = w32.astype(FP8)
        lo = (w32 - hi.astype(np.float32)).astype(FP8)
        out = np.stack([hi, lo])
        out = out.reshape(2, 4, 2, 128, 256).transpose(3, 0, 1, 2, 4)
        return np.ascontiguousarray(out)

    perm64 = np.concatenate([np.arange(0, 64, 2), np.arange(1, 64, 2)])

    half = D_HEAD // 2
    inv_freq = THETA ** (-(np.arange(half, dtype=np.float64) * 2.0 / D_HEAD))
    pos = np.arange(S, dtype=np.float64)
    ang = pos[None, :] * inv_freq[:, None]        # [32, S]
    angf = np.tile(ang, (4, 1))                   # [128, S], row p -> i = p % 32
    trig = np.ascontiguousarray(np.stack(
        [np.cos(angf), np.sin(angf)], axis=1).astype(ml_dtypes.bfloat16))

    spermT = np.zeros((128, 128), dtype=np.float32)
    for h in range(2):
        for i in range(32):
            spermT[h * 64 + 32 + i, h * 64 + i] = -1.0
            spermT[h * 64 + i, h * 64 + 32 + i] = 1.0

    kq = np.arange(128)
    mask128 = (np.arange(128)[None, :] >= kq[:, None]).astype(ml_dtypes.bfloat16)

    # per-batch x, shared by that batch's 4 cores
    xparts = []
    for b in range(B):
        xT = np.ascontiguousarray(x[b].T)         # [1024, 2048]
        xh8 = xT.astype(FP8)
        xl8 = (xT - xh8.astype(np.float32)).astype(FP8)
        xparts.append((xh8, xl8))

    shared = dict(trig=trig, sperm=spermT, mask128=mask128)

    in_maps = []
    for core in range(N_CORES):
        b, g = divmod(core, 4)
        rows = slice(256 * g, 256 * (g + 1))

        def permqk(w):
            # [256 rows] -> de-interleaved per 64-block, -> [1024, 256]
            wc = w[rows]
            blocks = [wc[64 * i:64 * (i + 1)][perm64] for i in range(4)]
            return np.ascontiguousarray(np.concatenate(blocks).T)

        woc = np.stack([
            np.ascontiguousarray(ow[:, 256 * g + 128 * hp:
                                    256 * g + 128 * (hp + 1)].T) / 32.0
            for hp in range(HP)], axis=1)          # [128, 2, 1024]

        in_maps.append(dict(
            shared,
            xh8=xparts[b][0], xl8=xparts[b][1],
            wq8=w8pair(permqk(qw)),
            wk8=w8pair(permqk(kw)),
            wv8=w8pair(np.ascontiguousarray(vw[rows].T)),
            wo=np.ascontiguousarray(woc),
        ))
    return in_maps


def kernel(q_weight, k_weight, v_weight, o_weight, in_features):
    rt = _build()
    in_maps = _host_inputs(q_weight, k_weight, v_weight, o_weight, in_features)
    res = rt["run_bass_kernel_spmd"](
        rt["nc"], in_maps, core_ids=list(range(N_CORES)),
    )
    y = np.zeros((B, S, D_MODEL), dtype=np.float32)
    for core in range(N_CORES):
        b = core // 4
        y[b] += np.asarray(res.results[core]["y"], dtype=np.float32)
    return y
